# revision 28
# baseline (speedup 1.0000x reference)
"""Causal attention (B=4, Sq=Sk=2048, D=1024, f32) on 8 TRN2 NeuronCores.

Strategy: pure data-parallel (no collectives). Each core handles one
(batch, half) shard: batch b = core//2, and half of the query rows of
that batch, chosen as an interleaving of 128-row tiles that balances
the causal workload. All 8 cores run the same program (SPMD); per-core
variation (which query rows, causal mask offsets) is carried entirely
in the data.

Per-core schedule: 8 query tiles (slots) of 128 rows; slot s covers
keys [0, 256*(s+1)) in uniform 256-key stages against kt/v stored as
256-key blocks (all DMA transfers contiguous).  Slots 0-3 (phase A)
run interleaved by round — all their [0,256) stages first — so the PE
has ~18 us of work available from the first ~3 MB of DMA, keeping the
head dense while the HBM stream (chip-bandwidth-bound with all 8 cores
loading) catches up.  Slots 4-7 (phase B) then run sequentially, each
accumulating P^T V directly in PSUM across its stages, which removes
the SBUF accumulate round-trips through the vector engine.

Compute: S = Q K^T via bf16 matmuls on host-pre-transposed layouts
(fp8 was measured at rel-err 3.2e-2 — over the 2e-2 budget — so bf16
it is); softmax without max-subtraction (logits S/32 ~ N(0,1)) with
the row-sum fused into the exp activation (accum_out); P^T via TensorE
transpose; P^T V into PSUM.  The (S, exp, transpose, PV) chain is
software-pipelined (T lags S by 2 stages, PV by 3) so the first exp is
not needed until the scalar queue has drained its early DMA issues and
ACT table load.  Pre-warm matmuls (memset stationary, no data deps)
bridge from the framework preamble to first data so the PE HAM clock
gate reaches 2.4 GHz with no idle re-throttle gap.  Loads are split by
first-need across the two HWDGE rings (sync + scalar queues; scalar
stops issuing before the exps start); phase-A stores issue from the
gpsimd (software-DGE) queue since sync is still issuing loads, and
slot-final PV runs d-halves outer so the first half's normalize+store
overlaps the second half's matmuls.
"""

import os
import numpy as np
import ml_dtypes

B, SQ, SK, D = 4, 2048, 2048, 1024
NCORES = 8
P = 128                      # partitions / tile rows
NDC = D // P                 # 8 d-chunks of 128
KB = 256                     # key block (stage width)
NKB = SK // KB               # 8 key blocks
NSLOT = 8                    # query tiles per core
# query-tile (128-row) indices of the batch handled by core parity j,
# ordered by slot (ascending causal need); complement pairs sum equally.
TILES_J0 = [0, 3, 5, 6, 8, 11, 13, 14]
TILES_J1 = [1, 2, 4, 7, 9, 10, 12, 15]
NEG = -1.0e9
SCALE = 1.0 / 32.0           # 1/sqrt(D)
NWARM = 36                   # pre-warm matmuls (256 cols each)
APH_SLOTS = 6                # slots 0..5 interleave (phase A); rest serial

_CACHE = {}


def _build_stages():
    """(s, k, last) stage list: phase A interleaved by round, phase B
    slot-sequential."""
    stages = []
    for k in range(APH_SLOTS):
        for s in range(k, APH_SLOTS):
            stages.append((s, k, k == s))
    for s in range(APH_SLOTS, NSLOT):
        for k in range(s + 1):
            stages.append((s, k, k == s))
    return stages


def _build_nc():
    import concourse.bacc as bacc
    import concourse.tile as tile
    import concourse.mybir as mybir
    from concourse.masks import make_identity

    dt = mybir.dt
    nc = bacc.Bacc("TRN2", target_bir_lowering=False, debug=False,
                   num_devices=NCORES, enable_partition_id=False)

    qt_ext = nc.dram_tensor("qt", [NSLOT, P, NDC, P], dt.bfloat16,
                            kind="ExternalInput").ap()
    kt_ext = nc.dram_tensor("kt", [NKB, P, NDC, KB], dt.bfloat16,
                            kind="ExternalInput").ap()
    v_ext = nc.dram_tensor("v", [NKB, P, KB // P, D], dt.bfloat16,
                           kind="ExternalInput").ap()
    thr_ext = nc.dram_tensor("thr", [P, NSLOT], dt.float32,
                             kind="ExternalInput").ap()
    out_ext = nc.dram_tensor("out", [NSLOT * P, D], dt.bfloat16,
                             kind="ExternalOutput").ap()

    stages = _build_stages()
    n = len(stages)
    # slot -> stage indices; phase-A prescale needs the second-to-last
    # stage's o_acc add (emitted at idx+3) to retire before the last
    # stage's prescale (emitted at idx+2)
    sidx = {s: [i for i, st in enumerate(stages) if st[0] == s]
            for s in range(NSLOT)}
    prescale_ok = {s: len(sidx[s]) > 1 and sidx[s][-2] + 1 < sidx[s][-1]
                   for s in range(APH_SLOTS)}

    with tile.TileContext(nc) as tc:
        with tc.tile_pool(name="big", bufs=1) as big, \
             tc.tile_pool(name="work", bufs=6) as work, \
             tc.tile_pool(name="acc", bufs=2) as acc, \
             tc.tile_pool(name="spsum", bufs=3, space="PSUM") as spsum, \
             tc.tile_pool(name="tpsum", bufs=1, space="PSUM") as tpsum, \
             tc.tile_pool(name="opsum", bufs=2, space="PSUM") as opsum:

            qt_sb = big.tile([P, NSLOT, NDC, P], dt.bfloat16)
            kt_sb = big.tile([P, NKB, NDC, KB], dt.bfloat16)
            v_sb = big.tile([P, NKB, KB // P, D], dt.bfloat16)
            mask_sb = big.tile([P, NSLOT, KB], dt.bfloat16)
            thr_sb = big.tile([P, NSLOT], dt.float32)
            m0i = big.tile([P, KB], dt.int32)
            m0f = big.tile([P, KB], dt.float32)
            rsums = big.tile([P, NSLOT, NSLOT], dt.float32)
            o_acc = big.tile([P, APH_SLOTS, D], dt.float32)
            ident = big.tile([P, P], dt.bfloat16)
            make_identity(nc, ident[:])
            # on-device causal masks: mask[p, s, f] = -1e9 where
            # (f - p) > thr[s], thr[s] = 128*gq_s - 256*s (per-core data).
            nc.gpsimd.iota(m0i[:], pattern=[[1, KB]], base=0,
                           channel_multiplier=-1)
            nc.vector.tensor_copy(m0f[:], m0i[:])

            # Loads split by first-need across the two HWDGE rings.  The
            # scalar (ACT) queue takes only the first three — it must drain
            # its DMA issues + ACT table load before the first exp.  Phase A
            # rounds reuse each kt/v block across all six slots, so after
            # thr/qt0/ktb0/vb0 the early stream only needs one qt per stage
            # and one kt/v block pair per ~11 us round.
            def qt_one(s):
                return (qt_sb[:, s:s + 1],
                        qt_ext[s:s + 1].rearrange("s p c q -> p s c q"))

            # scalar ring: just ktb0 + vb0, then the ACT table load and the
            # exp stream; everything else rides the sync ring in need order.
            nc.scalar.dma_start(kt_sb[:, 0], kt_ext[0])
            nc.sync.dma_start(thr_sb[:], thr_ext)
            nc.sync.dma_start(*qt_one(0))
            nc.scalar.dma_start(v_sb[:, 0], v_ext[0])
            # NOTE: no further scalar DMAs — exps follow on that queue
            nc.sync.dma_start(*qt_one(1))
            nc.sync.dma_start(*qt_one(2))
            nc.sync.dma_start(*qt_one(3))
            nc.sync.dma_start(kt_sb[:, 1], kt_ext[1])
            nc.sync.dma_start(*qt_one(4))
            nc.sync.dma_start(v_sb[:, 1], v_ext[1])
            nc.sync.dma_start(*qt_one(5))
            nc.sync.dma_start(kt_sb[:, 2], kt_ext[2])
            nc.sync.dma_start(v_sb[:, 2], v_ext[2])
            nc.sync.dma_start(kt_sb[:, 3], kt_ext[3])
            nc.sync.dma_start(v_sb[:, 3], v_ext[3])
            nc.sync.dma_start(kt_sb[:, 4], kt_ext[4])
            nc.sync.dma_start(v_sb[:, 4], v_ext[4])
            nc.sync.dma_start(kt_sb[:, 5], kt_ext[5])
            nc.sync.dma_start(v_sb[:, 5], v_ext[5])
            nc.sync.dma_start(*qt_one(6))
            nc.sync.dma_start(kt_sb[:, 6], kt_ext[6])
            nc.sync.dma_start(v_sb[:, 6], v_ext[6])
            nc.sync.dma_start(*qt_one(7))
            nc.sync.dma_start(kt_sb[:, 7], kt_ext[7])
            nc.sync.dma_start(v_sb[:, 7], v_ext[7])

            # Pre-warm the PE HAM clock gate from right after the preamble
            # (memset stationary — no dependency on the ident/iota chain)
            # until the first data lands, so the real stream starts at or
            # near 2.4 GHz with no re-throttling idle gap.
            warm_ps = spsum.tile([P, KB], dt.float32, tag="s")
            scratch = big.tile([P, KB], dt.bfloat16)
            warm_st = big.tile([P, P], dt.bfloat16)
            nc.vector.memset(scratch[:, 0:1], 0.0)
            nc.vector.memset(warm_st[:, 0:1], 0.0)
            for w in range(NWARM):
                nc.tensor.matmul(warm_ps[:], warm_st[:], scratch[:],
                                 start=True, stop=True)

            state = {}
            recips = {}
            slot_ps = {}

            def emit_s(i):
                s, k, last = stages[i]
                s_ps = spsum.tile([P, KB], dt.float32, tag="s")
                for c in range(NDC):
                    nc.tensor.matmul(s_ps[:], qt_sb[:, s, c], kt_sb[:, k, c],
                                     start=(c == 0), stop=(c == NDC - 1))
                if last:
                    # build this slot's causal mask just-in-time
                    nc.vector.tensor_scalar(mask_sb[:, s], m0f[:],
                                            thr_sb[:, s:s + 1], NEG,
                                            op0=mybir.AluOpType.is_gt,
                                            op1=mybir.AluOpType.mult)
                    nc.vector.tensor_tensor(s_ps[:], s_ps[:], mask_sb[:, s],
                                            op=mybir.AluOpType.add)
                p_t = work.tile([P, KB], dt.bfloat16, tag="p")
                nc.scalar.activation(p_t[:], s_ps[:],
                                     mybir.ActivationFunctionType.Exp,
                                     scale=SCALE,
                                     accum_out=rsums[:, s, k:k + 1])
                state[("p", i)] = p_t

            def emit_t(i):
                s, k, last = stages[i]
                p_t = state.pop(("p", i))
                pt_ps = tpsum.tile([P, KB // P, P], dt.bfloat16, tag="tp")
                for c in range(KB // P):
                    nc.tensor.transpose(pt_ps[:, c], p_t[:, c * P:(c + 1) * P],
                                        ident[:])
                pt_t = work.tile([P, KB // P, P], dt.bfloat16, tag="pt")
                nc.vector.tensor_copy(pt_t[:], pt_ps[:])
                state[("pt", i)] = pt_t
                if last:
                    emit_recip(s)
                    if s < APH_SLOTS and prescale_ok[s]:
                        # fold 1/rowsum into o_acc off the critical path
                        nc.vector.tensor_scalar(o_acc[:, s], o_acc[:, s],
                                                recips[s][:], None,
                                                op0=mybir.AluOpType.mult)

            def emit_recip(s):
                nk = s + 1
                recip = work.tile([P, 1], dt.float32, name=f"recip{s}",
                                  tag="recip")
                if nk == 1:
                    nc.vector.reciprocal(recip[:], rsums[:, s, :1])
                else:
                    rtot = work.tile([P, 1], dt.float32, tag="rtot")
                    nc.vector.tensor_reduce(rtot[:], rsums[:, s, :nk],
                                            axis=mybir.AxisListType.X,
                                            op=mybir.AluOpType.add)
                    nc.vector.reciprocal(recip[:], rtot[:])
                recips[s] = recip

            def emit_store(s, o_sb, h, final=False):
                # final stage: both HWDGE queues (scalar's exps are done);
                # phase A: gpsimd (sync is still issuing loads); phase B:
                # sync + gpsimd
                hs = slice(h * (D // 2), (h + 1) * (D // 2))
                if final:
                    eng = nc.sync if h == 0 else nc.scalar
                elif s < APH_SLOTS or h == 1:
                    eng = nc.gpsimd
                else:
                    eng = nc.sync
                eng.dma_start(out_ext[s * P:(s + 1) * P, hs], o_sb[:, hs])

            def emit_pv(i):
                s, k, last = stages[i]
                pt_t = state.pop(("pt", i))
                nch = KB // P
                phase_a = s < APH_SLOTS
                if phase_a:
                    o_ps = opsum.tile([P, D], dt.float32, tag="o")
                    first, fin = True, True
                else:
                    if k == 0:
                        slot_ps[s] = opsum.tile([P, D], dt.float32,
                                                name=f"ops{s}", tag="o")
                    o_ps = slot_ps[s]
                    first, fin = (k == 0), last
                if last:
                    # d-halves outer so half 0 finishes nch matmuls early
                    # and its normalize+store overlaps half 1
                    for h in range(2):
                        hs = slice(h * (D // 2), (h + 1) * (D // 2))
                        for c in range(nch):
                            kc = (k * KB) // P + c
                            nc.tensor.matmul(
                                o_ps[:, hs], pt_t[:, c],
                                v_sb[:, kc // nch, kc % nch, hs],
                                start=(first and c == 0),
                                stop=(fin and c == nch - 1))
                else:
                    for c in range(nch):
                        kc = (k * KB) // P + c
                        for h in range(2):
                            nc.tensor.matmul(
                                o_ps[:, h * (D // 2):(h + 1) * (D // 2)],
                                pt_t[:, c],
                                v_sb[:, kc // nch, kc % nch,
                                     h * (D // 2):(h + 1) * (D // 2)],
                                start=(first and c == 0),
                                stop=(fin and c == nch - 1))

                # (gpsimd cannot read PSUM, so all accumulates stay on DVE)
                aeng = nc.vector
                fin_stage = (i == n - 1)
                if last:
                    if fin_stage:
                        # separate half-tiles so the DVE (h0) and ACT (h1)
                        # normalizes carry no false write-order dependency
                        o_sb = acc.tile([P, D], dt.bfloat16, tag="o_sb")
                        o_sb1 = acc.tile([P, D // 2], dt.bfloat16,
                                         tag="o_sb1")
                    else:
                        o_sb = acc.tile([P, D], dt.bfloat16, tag="o_sb")
                    if phase_a and s > 0 and not prescale_ok[s]:
                        # adjacent second-to-last stage: no room to
                        # prescale; fold o_ps into o_acc, then scale
                        aeng.tensor_tensor(o_acc[:, s], o_acc[:, s],
                                           o_ps[:],
                                           op=mybir.AluOpType.add)
                        for h in range(2):
                            hs = slice(h * (D // 2), (h + 1) * (D // 2))
                            nc.vector.tensor_scalar(
                                o_sb[:, hs], o_acc[:, s, hs], recips[s][:],
                                None, op0=mybir.AluOpType.mult)
                            emit_store(s, o_sb, h, fin_stage)
                    elif phase_a and s > 0:
                        for h in range(2):
                            hs = slice(h * (D // 2), (h + 1) * (D // 2))
                            nc.vector.scalar_tensor_tensor(
                                o_sb[:, hs], o_ps[:, hs], recips[s][:],
                                o_acc[:, s, hs],
                                op0=mybir.AluOpType.mult,
                                op1=mybir.AluOpType.add)
                            emit_store(s, o_sb, h, fin_stage)
                    else:
                        # s == 0 or phase B: o_ps holds the whole sum.  On
                        # the very last stage the scalar engine (done with
                        # exps) normalizes half 1 in parallel with the DVE.
                        for h in range(2):
                            hs = slice(h * (D // 2), (h + 1) * (D // 2))
                            if fin_stage and h == 1:
                                nc.scalar.activation(
                                    o_sb1[:], o_ps[:, hs],
                                    mybir.ActivationFunctionType.Copy,
                                    scale=recips[s][:])
                                nc.scalar.dma_start(
                                    out_ext[s * P:(s + 1) * P, hs], o_sb1[:])
                                continue
                            nc.vector.tensor_scalar(
                                o_sb[:, hs], o_ps[:, hs], recips[s][:],
                                None, op0=mybir.AluOpType.mult)
                            emit_store(s, o_sb, h, fin_stage)
                    if not phase_a:
                        slot_ps.pop(s)
                    return
                if phase_a:
                    if k == 0:
                        aeng.tensor_copy(o_acc[:, s], o_ps[:])
                    else:
                        aeng.tensor_tensor(o_acc[:, s], o_acc[:, s],
                                           o_ps[:],
                                           op=mybir.AluOpType.add)

            for i in range(n + 3):
                if i < n:
                    emit_s(i)
                if 2 <= i <= n + 1:
                    emit_t(i - 2)
                if i >= 3:
                    emit_pv(i - 3)

    nc.compile()
    return nc


def _install_axon_hooks_shim():
    """concourse's trace path imports antenv.axon_hooks, which this image
    lacks; provide it (backed by the libaxon ctypes hook when available)
    so run_bass_kernel_spmd(trace=True) degrades gracefully."""
    import sys, types
    if "antenv.axon_hooks" in sys.modules:
        return
    hook = None
    try:
        from trn_agent_boot.trn_boot import _ntff_profile_via_ctypes
        hook = _ntff_profile_via_ctypes("/opt/axon/libaxon_pjrt.so")
    except Exception:
        hook = None
    mod = types.ModuleType("antenv.axon_hooks")
    mod.get_axon_ntff_profile_hook = lambda: hook
    mod.set_axon_ntff_profile_hook = lambda h: None
    sys.modules["antenv.axon_hooks"] = mod


def _get_nc():
    if "nc" not in _CACHE:
        os.environ.setdefault("JAX_COMPILATION_CACHE_DIR", "/tmp/jax_comp_cache")
        try:
            import jax
            jax.config.update("jax_compilation_cache_dir", "/tmp/jax_comp_cache")
            jax.config.update("jax_persistent_cache_min_entry_size_bytes", -1)
            jax.config.update("jax_persistent_cache_min_compile_time_secs", 0)
        except Exception:
            pass
        _install_axon_hooks_shim()
        _CACHE["nc"] = _build_nc()
    return _CACHE["nc"]


def _host_thr(tiles):
    """[128, NSLOT] per-slot causal thresholds: mask where (f-p) > thr[s]."""
    thr = np.empty((P, NSLOT), np.float32)
    for s in range(NSLOT):
        thr[:, s] = P * tiles[s] - KB * s
    return thr


def make_in_maps(query, key, value):
    query = np.asarray(query, np.float32)
    key = np.asarray(key, np.float32)
    value = np.asarray(value, np.float32)
    in_maps = []
    for core in range(NCORES):
        b, j = divmod(core, 2)
        tiles = TILES_J0 if j == 0 else TILES_J1
        qrows = np.concatenate([query[b, P * t:P * (t + 1)] for t in tiles])
        # qt[s, p, c, q] = qrows[s*128+q, c*128+p]
        qt = np.ascontiguousarray(
            qrows.astype(ml_dtypes.bfloat16)
            .reshape(NSLOT, P, NDC, P).transpose(0, 3, 2, 1))
        # kt[blk, p, c, k] = key[b, blk*256+k, c*128+p]
        kt = np.ascontiguousarray(
            key[b].astype(ml_dtypes.bfloat16)
            .reshape(NKB, KB, NDC, P).transpose(0, 3, 2, 1))
        # v[blk, p, kc, d] = value[b, blk*256 + kc*128 + p, d]
        v = np.ascontiguousarray(
            value[b].astype(ml_dtypes.bfloat16)
            .reshape(NKB, KB // P, P, D).transpose(0, 2, 1, 3))
        in_maps.append({
            "qt": qt,
            "kt": kt,
            "v": v,
            "thr": _host_thr(tiles),
        })
    return in_maps


def assemble(results):
    out = np.empty((B, SQ, D), np.float32)
    for core in range(NCORES):
        b, j = divmod(core, 2)
        tiles = TILES_J0 if j == 0 else TILES_J1
        o = np.asarray(results[core]["out"], dtype=np.float32)
        for s, t in enumerate(tiles):
            out[b, P * t:P * (t + 1)] = o[P * s:P * (s + 1)]
    return out


def _get_runner(nc):
    """Build once: a jitted SPMD executable over the 8 axon devices
    (mirrors bass2jax.run_bass_via_pjrt, but cached across kernel() calls
    so repeat calls skip tracing/compilation)."""
    if "runner" in _CACHE:
        return _CACHE["runner"]
    import jax
    import concourse.mybir as mybir
    from concourse import bass2jax
    from jax.sharding import Mesh, PartitionSpec
    from jax.experimental.shard_map import shard_map
    import numpy as _np

    bass2jax.install_neuronx_cc_hook()
    partition_name = (nc.partition_id_tensor.name
                      if nc.partition_id_tensor else None)
    in_names, out_names, out_avals, zero_outs = [], [], [], []
    for alloc in nc.m.functions[0].allocations:
        if not isinstance(alloc, mybir.MemoryLocationSet):
            continue
        name = alloc.memorylocations[0].name
        if alloc.kind == "ExternalInput":
            if name != partition_name:
                in_names.append(name)
        elif alloc.kind == "ExternalOutput":
            out_names.append(name)
            shape = tuple(alloc.tensor_shape)
            dtype = mybir.dt.np(alloc.dtype)
            out_avals.append(jax.core.ShapedArray(shape, dtype))
            zero_outs.append(_np.zeros(shape, dtype))
    n_params = len(in_names)
    all_names = in_names + out_names
    if partition_name is not None:
        all_names = all_names + [partition_name]

    def _body(*args):
        operands = list(args)
        if partition_name is not None:
            operands.append(bass2jax.partition_id_tensor())
        outs = bass2jax._bass_exec_p.bind(
            *operands,
            out_avals=tuple(out_avals),
            in_names=tuple(all_names),
            out_names=tuple(out_names),
            lowering_input_output_aliases=(),
            sim_require_finite=True,
            sim_require_nnan=True,
            nc=nc,
        )
        return tuple(outs)

    devices = jax.devices()[:NCORES]
    mesh = Mesh(_np.asarray(devices), ("core",))
    n_outs = len(out_names)
    sharded = jax.jit(
        shard_map(_body, mesh=mesh,
                  in_specs=(PartitionSpec("core"),) * (n_params + n_outs),
                  out_specs=(PartitionSpec("core"),) * n_outs,
                  check_rep=False),
        donate_argnums=tuple(range(n_params, n_params + n_outs)),
        keep_unused=True,
    )
    _CACHE["runner"] = (sharded, in_names, out_names, out_avals, zero_outs)
    return _CACHE["runner"]


def kernel(query, key, value, _run_kwargs=None):
    import numpy as _np
    nc = _get_nc()
    in_maps = make_in_maps(query, key, value)
    if _run_kwargs is not None:
        # profiling path for test.py
        from concourse.bass_utils import run_bass_kernel_spmd
        res = run_bass_kernel_spmd(nc, in_maps, list(range(NCORES)),
                                   **dict(_run_kwargs))
        _CACHE["last_result"] = res
        return assemble(res.results)
    sharded, in_names, out_names, out_avals, zero_outs = _get_runner(nc)
    concat_in = [
        _np.concatenate([m[name] for m in in_maps], axis=0)
        for name in in_names
    ]
    concat_zeros = [
        _np.zeros((NCORES * z.shape[0], *z.shape[1:]), z.dtype)
        for z in zero_outs
    ]
    out_arrs = sharded(*concat_in, *concat_zeros)
    results = [
        {name: _np.asarray(out_arrs[i]).reshape(NCORES, *out_avals[i].shape)[c]
         for i, name in enumerate(out_names)}
        for c in range(NCORES)
    ]
    return assemble(results)


# revision 29
# speedup vs baseline: 1.0158x; 1.0158x over previous
"""Causal attention (B=4, Sq=Sk=2048, D=1024, f32) on 8 TRN2 NeuronCores.

Strategy: pure data-parallel (no collectives). Each core handles one
(batch, half) shard: batch b = core//2, and half of the query rows of
that batch, chosen as an interleaving of 128-row tiles that balances
the causal workload. All 8 cores run the same program (SPMD); per-core
variation (which query rows, causal mask offsets) is carried entirely
in the data.

Per-core schedule: 8 query tiles (slots) of 128 rows; slot s covers
keys [0, 256*(s+1)) in uniform 256-key stages against kt/v stored as
256-key blocks (all DMA transfers contiguous).  Slots 0-3 (phase A)
run interleaved by round — all their [0,256) stages first — so the PE
has ~18 us of work available from the first ~3 MB of DMA, keeping the
head dense while the HBM stream (chip-bandwidth-bound with all 8 cores
loading) catches up.  Slots 4-7 (phase B) then run sequentially, each
accumulating P^T V directly in PSUM across its stages, which removes
the SBUF accumulate round-trips through the vector engine.

Compute: S = Q K^T via bf16 matmuls on host-pre-transposed layouts
(fp8 was measured at rel-err 3.2e-2 — over the 2e-2 budget — so bf16
it is); softmax without max-subtraction (logits S/32 ~ N(0,1)) with
the row-sum fused into the exp activation (accum_out); P^T via TensorE
transpose; P^T V into PSUM.  The (S, exp, transpose, PV) chain is
software-pipelined (T lags S by 2 stages, PV by 3) so the first exp is
not needed until the scalar queue has drained its early DMA issues and
ACT table load.  Pre-warm matmuls (memset stationary, no data deps)
bridge from the framework preamble to first data so the PE HAM clock
gate reaches 2.4 GHz with no idle re-throttle gap.  Loads are split by
first-need across the two HWDGE rings (sync + scalar queues; scalar
stops issuing before the exps start); phase-A stores issue from the
gpsimd (software-DGE) queue since sync is still issuing loads, and
slot-final PV runs d-halves outer so the first half's normalize+store
overlaps the second half's matmuls.
"""

import os
import numpy as np
import ml_dtypes

B, SQ, SK, D = 4, 2048, 2048, 1024
NCORES = 8
P = 128                      # partitions / tile rows
NDC = D // P                 # 8 d-chunks of 128
KB = 256                     # key block (stage width)
NKB = SK // KB               # 8 key blocks
NSLOT = 8                    # query tiles per core
# query-tile (128-row) indices of the batch handled by core parity j,
# ordered by slot (ascending causal need); complement pairs sum equally.
TILES_J0 = [0, 3, 5, 6, 8, 11, 13, 14]
TILES_J1 = [1, 2, 4, 7, 9, 10, 12, 15]
NEG = -1.0e9
SCALE = 1.0 / 32.0           # 1/sqrt(D)
NWARM = 36                   # pre-warm matmuls (256 cols each)
APH_SLOTS = 6                # slots 0..5 interleave (phase A); rest serial

_CACHE = {}


def _build_stages():
    """(s, k, last) stage list: phase A interleaved by round, phase B
    slot-sequential."""
    stages = []
    for k in range(APH_SLOTS):
        for s in range(k, APH_SLOTS):
            stages.append((s, k, k == s))
    for s in range(APH_SLOTS, NSLOT):
        for k in range(s + 1):
            stages.append((s, k, k == s))
    return stages


def _build_nc():
    import concourse.bacc as bacc
    import concourse.tile as tile
    import concourse.mybir as mybir
    from concourse.masks import make_identity

    dt = mybir.dt
    nc = bacc.Bacc("TRN2", target_bir_lowering=False, debug=False,
                   num_devices=NCORES, enable_partition_id=False)

    qt_ext = nc.dram_tensor("qt", [NSLOT, P, NDC, P], dt.bfloat16,
                            kind="ExternalInput").ap()
    kt_ext = nc.dram_tensor("kt", [NKB, P, NDC, KB], dt.bfloat16,
                            kind="ExternalInput").ap()
    v_ext = nc.dram_tensor("v", [NKB, P, KB // P, D], dt.bfloat16,
                           kind="ExternalInput").ap()
    thr_ext = nc.dram_tensor("thr", [P, NSLOT], dt.float32,
                             kind="ExternalInput").ap()
    out_ext = nc.dram_tensor("out", [NSLOT * P, D], dt.bfloat16,
                             kind="ExternalOutput").ap()

    stages = _build_stages()
    n = len(stages)
    # slot -> stage indices; phase-A prescale needs the second-to-last
    # stage's o_acc add (emitted at idx+3) to retire before the last
    # stage's prescale (emitted at idx+2)
    sidx = {s: [i for i, st in enumerate(stages) if st[0] == s]
            for s in range(NSLOT)}
    prescale_ok = {s: len(sidx[s]) > 1 and sidx[s][-2] + 1 < sidx[s][-1]
                   for s in range(APH_SLOTS)}

    with tile.TileContext(nc) as tc:
        with tc.tile_pool(name="big", bufs=1) as big, \
             tc.tile_pool(name="work", bufs=6) as work, \
             tc.tile_pool(name="acc", bufs=2) as acc, \
             tc.tile_pool(name="spsum", bufs=3, space="PSUM") as spsum, \
             tc.tile_pool(name="tpsum", bufs=1, space="PSUM") as tpsum, \
             tc.tile_pool(name="opsum", bufs=2, space="PSUM") as opsum:

            qt_sb = big.tile([P, NSLOT, NDC, P], dt.bfloat16)
            kt_sb = big.tile([P, NKB, NDC, KB], dt.bfloat16)
            v_sb = big.tile([P, NKB, KB // P, D], dt.bfloat16)
            mask_sb = big.tile([P, NSLOT, KB], dt.bfloat16)
            thr_sb = big.tile([P, NSLOT], dt.float32)
            m0i = big.tile([P, KB], dt.int32)
            m0f = big.tile([P, KB], dt.float32)
            rsums = big.tile([P, NSLOT, NSLOT], dt.float32)
            o_acc = big.tile([P, APH_SLOTS, D], dt.float32)
            ident = big.tile([P, P], dt.bfloat16)
            make_identity(nc, ident[:])
            # on-device causal masks: mask[p, s, f] = -1e9 where
            # (f - p) > thr[s], thr[s] = 128*gq_s - 256*s (per-core data).
            nc.gpsimd.iota(m0i[:], pattern=[[1, KB]], base=0,
                           channel_multiplier=-1)
            nc.vector.tensor_copy(m0f[:], m0i[:])

            # Loads split by first-need across the two HWDGE rings.  The
            # scalar (ACT) queue takes only the first three — it must drain
            # its DMA issues + ACT table load before the first exp.  Phase A
            # rounds reuse each kt/v block across all six slots, so after
            # thr/qt0/ktb0/vb0 the early stream only needs one qt per stage
            # and one kt/v block pair per ~11 us round.
            def qt_one(s):
                return (qt_sb[:, s:s + 1],
                        qt_ext[s:s + 1].rearrange("s p c q -> p s c q"))

            # scalar ring: just ktb0 + vb0, then the ACT table load and the
            # exp stream; everything else rides the sync ring in need order.
            nc.scalar.dma_start(kt_sb[:, 0], kt_ext[0])
            nc.sync.dma_start(thr_sb[:], thr_ext)
            nc.sync.dma_start(*qt_one(0))
            nc.scalar.dma_start(v_sb[:, 0], v_ext[0])
            nc.scalar.dma_start(*qt_one(1))
            # NOTE: no further scalar DMAs — exps follow on that queue
            nc.sync.dma_start(*qt_one(2))
            nc.sync.dma_start(*qt_one(3))
            nc.sync.dma_start(kt_sb[:, 1], kt_ext[1])
            nc.sync.dma_start(*qt_one(4))
            nc.sync.dma_start(v_sb[:, 1], v_ext[1])
            nc.sync.dma_start(*qt_one(5))
            nc.sync.dma_start(kt_sb[:, 2], kt_ext[2])
            nc.sync.dma_start(v_sb[:, 2], v_ext[2])
            nc.sync.dma_start(kt_sb[:, 3], kt_ext[3])
            nc.sync.dma_start(v_sb[:, 3], v_ext[3])
            nc.sync.dma_start(kt_sb[:, 4], kt_ext[4])
            nc.sync.dma_start(v_sb[:, 4], v_ext[4])
            nc.sync.dma_start(kt_sb[:, 5], kt_ext[5])
            nc.sync.dma_start(v_sb[:, 5], v_ext[5])
            nc.sync.dma_start(*qt_one(6))
            nc.sync.dma_start(kt_sb[:, 6], kt_ext[6])
            nc.sync.dma_start(v_sb[:, 6], v_ext[6])
            nc.sync.dma_start(*qt_one(7))
            nc.sync.dma_start(kt_sb[:, 7], kt_ext[7])
            nc.sync.dma_start(v_sb[:, 7], v_ext[7])

            # Pre-warm the PE HAM clock gate from right after the preamble
            # (memset stationary — no dependency on the ident/iota chain)
            # until the first data lands, so the real stream starts at or
            # near 2.4 GHz with no re-throttling idle gap.
            warm_ps = spsum.tile([P, KB], dt.float32, tag="s")
            scratch = big.tile([P, KB], dt.bfloat16)
            warm_st = big.tile([P, P], dt.bfloat16)
            nc.vector.memset(scratch[:, 0:1], 0.0)
            nc.vector.memset(warm_st[:, 0:1], 0.0)
            for w in range(NWARM):
                nc.tensor.matmul(warm_ps[:], warm_st[:], scratch[:],
                                 start=True, stop=True)

            state = {}
            recips = {}
            slot_ps = {}

            def emit_s(i):
                s, k, last = stages[i]
                s_ps = spsum.tile([P, KB], dt.float32, tag="s")
                for c in range(NDC):
                    nc.tensor.matmul(s_ps[:], qt_sb[:, s, c], kt_sb[:, k, c],
                                     start=(c == 0), stop=(c == NDC - 1))
                if last:
                    # build this slot's causal mask just-in-time
                    nc.vector.tensor_scalar(mask_sb[:, s], m0f[:],
                                            thr_sb[:, s:s + 1], NEG,
                                            op0=mybir.AluOpType.is_gt,
                                            op1=mybir.AluOpType.mult)
                    nc.vector.tensor_tensor(s_ps[:], s_ps[:], mask_sb[:, s],
                                            op=mybir.AluOpType.add)
                p_t = work.tile([P, KB], dt.bfloat16, tag="p")
                nc.scalar.activation(p_t[:], s_ps[:],
                                     mybir.ActivationFunctionType.Exp,
                                     scale=SCALE,
                                     accum_out=rsums[:, s, k:k + 1])
                state[("p", i)] = p_t

            def emit_t(i):
                s, k, last = stages[i]
                p_t = state.pop(("p", i))
                pt_ps = tpsum.tile([P, KB // P, P], dt.bfloat16, tag="tp")
                for c in range(KB // P):
                    nc.tensor.transpose(pt_ps[:, c], p_t[:, c * P:(c + 1) * P],
                                        ident[:])
                pt_t = work.tile([P, KB // P, P], dt.bfloat16, tag="pt")
                nc.vector.tensor_copy(pt_t[:], pt_ps[:])
                state[("pt", i)] = pt_t
                if last:
                    emit_recip(s)
                    if s < APH_SLOTS and prescale_ok[s]:
                        # fold 1/rowsum into o_acc off the critical path
                        nc.vector.tensor_scalar(o_acc[:, s], o_acc[:, s],
                                                recips[s][:], None,
                                                op0=mybir.AluOpType.mult)

            def emit_recip(s):
                nk = s + 1
                recip = work.tile([P, 1], dt.float32, name=f"recip{s}",
                                  tag="recip")
                if nk == 1:
                    nc.vector.reciprocal(recip[:], rsums[:, s, :1])
                else:
                    rtot = work.tile([P, 1], dt.float32, tag="rtot")
                    nc.vector.tensor_reduce(rtot[:], rsums[:, s, :nk],
                                            axis=mybir.AxisListType.X,
                                            op=mybir.AluOpType.add)
                    nc.vector.reciprocal(recip[:], rtot[:])
                recips[s] = recip

            def emit_store(s, o_sb, h, final=False):
                # final stage: both HWDGE queues (scalar's exps are done);
                # phase A: gpsimd (sync is still issuing loads); phase B:
                # sync + gpsimd
                hs = slice(h * (D // 2), (h + 1) * (D // 2))
                if final:
                    eng = nc.sync if h == 0 else nc.scalar
                elif s < APH_SLOTS or h == 1:
                    eng = nc.gpsimd
                else:
                    eng = nc.sync
                eng.dma_start(out_ext[s * P:(s + 1) * P, hs], o_sb[:, hs])

            def emit_pv(i):
                s, k, last = stages[i]
                pt_t = state.pop(("pt", i))
                nch = KB // P
                phase_a = s < APH_SLOTS
                if phase_a:
                    o_ps = opsum.tile([P, D], dt.float32, tag="o")
                    first, fin = True, True
                else:
                    if k == 0:
                        slot_ps[s] = opsum.tile([P, D], dt.float32,
                                                name=f"ops{s}", tag="o")
                    o_ps = slot_ps[s]
                    first, fin = (k == 0), last
                if last:
                    # d-halves outer so half 0 finishes nch matmuls early
                    # and its normalize+store overlaps half 1
                    for h in range(2):
                        hs = slice(h * (D // 2), (h + 1) * (D // 2))
                        for c in range(nch):
                            kc = (k * KB) // P + c
                            nc.tensor.matmul(
                                o_ps[:, hs], pt_t[:, c],
                                v_sb[:, kc // nch, kc % nch, hs],
                                start=(first and c == 0),
                                stop=(fin and c == nch - 1))
                else:
                    for c in range(nch):
                        kc = (k * KB) // P + c
                        for h in range(2):
                            nc.tensor.matmul(
                                o_ps[:, h * (D // 2):(h + 1) * (D // 2)],
                                pt_t[:, c],
                                v_sb[:, kc // nch, kc % nch,
                                     h * (D // 2):(h + 1) * (D // 2)],
                                start=(first and c == 0),
                                stop=(fin and c == nch - 1))

                # (gpsimd cannot read PSUM, so all accumulates stay on DVE)
                aeng = nc.vector
                fin_stage = (i == n - 1)
                if last:
                    if fin_stage:
                        # separate half-tiles so the DVE (h0) and ACT (h1)
                        # normalizes carry no false write-order dependency
                        o_sb = acc.tile([P, D], dt.bfloat16, tag="o_sb")
                        o_sb1 = acc.tile([P, D // 2], dt.bfloat16,
                                         tag="o_sb1")
                    else:
                        o_sb = acc.tile([P, D], dt.bfloat16, tag="o_sb")
                    if phase_a and s > 0 and not prescale_ok[s]:
                        # adjacent second-to-last stage: no room to
                        # prescale; fold o_ps into o_acc, then scale
                        aeng.tensor_tensor(o_acc[:, s], o_acc[:, s],
                                           o_ps[:],
                                           op=mybir.AluOpType.add)
                        for h in range(2):
                            hs = slice(h * (D // 2), (h + 1) * (D // 2))
                            nc.vector.tensor_scalar(
                                o_sb[:, hs], o_acc[:, s, hs], recips[s][:],
                                None, op0=mybir.AluOpType.mult)
                            emit_store(s, o_sb, h, fin_stage)
                    elif phase_a and s > 0:
                        for h in range(2):
                            hs = slice(h * (D // 2), (h + 1) * (D // 2))
                            nc.vector.scalar_tensor_tensor(
                                o_sb[:, hs], o_ps[:, hs], recips[s][:],
                                o_acc[:, s, hs],
                                op0=mybir.AluOpType.mult,
                                op1=mybir.AluOpType.add)
                            emit_store(s, o_sb, h, fin_stage)
                    else:
                        # s == 0 or phase B: o_ps holds the whole sum.  On
                        # the very last stage the scalar engine (done with
                        # exps) normalizes half 1 in parallel with the DVE.
                        for h in range(2):
                            hs = slice(h * (D // 2), (h + 1) * (D // 2))
                            if fin_stage and h == 1:
                                nc.scalar.activation(
                                    o_sb1[:], o_ps[:, hs],
                                    mybir.ActivationFunctionType.Copy,
                                    scale=recips[s][:])
                                nc.scalar.dma_start(
                                    out_ext[s * P:(s + 1) * P, hs], o_sb1[:])
                                continue
                            nc.vector.tensor_scalar(
                                o_sb[:, hs], o_ps[:, hs], recips[s][:],
                                None, op0=mybir.AluOpType.mult)
                            emit_store(s, o_sb, h, fin_stage)
                    if not phase_a:
                        slot_ps.pop(s)
                    return
                if phase_a:
                    if k == 0:
                        aeng.tensor_copy(o_acc[:, s], o_ps[:])
                    else:
                        aeng.tensor_tensor(o_acc[:, s], o_acc[:, s],
                                           o_ps[:],
                                           op=mybir.AluOpType.add)

            for i in range(n + 3):
                if i < n:
                    emit_s(i)
                if 2 <= i <= n + 1:
                    emit_t(i - 2)
                if i >= 3:
                    emit_pv(i - 3)

    nc.compile()
    return nc


def _install_axon_hooks_shim():
    """concourse's trace path imports antenv.axon_hooks, which this image
    lacks; provide it (backed by the libaxon ctypes hook when available)
    so run_bass_kernel_spmd(trace=True) degrades gracefully."""
    import sys, types
    if "antenv.axon_hooks" in sys.modules:
        return
    hook = None
    try:
        from trn_agent_boot.trn_boot import _ntff_profile_via_ctypes
        hook = _ntff_profile_via_ctypes("/opt/axon/libaxon_pjrt.so")
    except Exception:
        hook = None
    mod = types.ModuleType("antenv.axon_hooks")
    mod.get_axon_ntff_profile_hook = lambda: hook
    mod.set_axon_ntff_profile_hook = lambda h: None
    sys.modules["antenv.axon_hooks"] = mod


def _get_nc():
    if "nc" not in _CACHE:
        os.environ.setdefault("JAX_COMPILATION_CACHE_DIR", "/tmp/jax_comp_cache")
        try:
            import jax
            jax.config.update("jax_compilation_cache_dir", "/tmp/jax_comp_cache")
            jax.config.update("jax_persistent_cache_min_entry_size_bytes", -1)
            jax.config.update("jax_persistent_cache_min_compile_time_secs", 0)
        except Exception:
            pass
        _install_axon_hooks_shim()
        _CACHE["nc"] = _build_nc()
    return _CACHE["nc"]


def _host_thr(tiles):
    """[128, NSLOT] per-slot causal thresholds: mask where (f-p) > thr[s]."""
    thr = np.empty((P, NSLOT), np.float32)
    for s in range(NSLOT):
        thr[:, s] = P * tiles[s] - KB * s
    return thr


def make_in_maps(query, key, value):
    query = np.asarray(query, np.float32)
    key = np.asarray(key, np.float32)
    value = np.asarray(value, np.float32)
    in_maps = []
    for core in range(NCORES):
        b, j = divmod(core, 2)
        tiles = TILES_J0 if j == 0 else TILES_J1
        qrows = np.concatenate([query[b, P * t:P * (t + 1)] for t in tiles])
        # qt[s, p, c, q] = qrows[s*128+q, c*128+p]
        qt = np.ascontiguousarray(
            qrows.astype(ml_dtypes.bfloat16)
            .reshape(NSLOT, P, NDC, P).transpose(0, 3, 2, 1))
        # kt[blk, p, c, k] = key[b, blk*256+k, c*128+p]
        kt = np.ascontiguousarray(
            key[b].astype(ml_dtypes.bfloat16)
            .reshape(NKB, KB, NDC, P).transpose(0, 3, 2, 1))
        # v[blk, p, kc, d] = value[b, blk*256 + kc*128 + p, d]
        v = np.ascontiguousarray(
            value[b].astype(ml_dtypes.bfloat16)
            .reshape(NKB, KB // P, P, D).transpose(0, 2, 1, 3))
        in_maps.append({
            "qt": qt,
            "kt": kt,
            "v": v,
            "thr": _host_thr(tiles),
        })
    return in_maps


def assemble(results):
    out = np.empty((B, SQ, D), np.float32)
    for core in range(NCORES):
        b, j = divmod(core, 2)
        tiles = TILES_J0 if j == 0 else TILES_J1
        o = np.asarray(results[core]["out"], dtype=np.float32)
        for s, t in enumerate(tiles):
            out[b, P * t:P * (t + 1)] = o[P * s:P * (s + 1)]
    return out


def _get_runner(nc):
    """Build once: a jitted SPMD executable over the 8 axon devices
    (mirrors bass2jax.run_bass_via_pjrt, but cached across kernel() calls
    so repeat calls skip tracing/compilation)."""
    if "runner" in _CACHE:
        return _CACHE["runner"]
    import jax
    import concourse.mybir as mybir
    from concourse import bass2jax
    from jax.sharding import Mesh, PartitionSpec
    from jax.experimental.shard_map import shard_map
    import numpy as _np

    bass2jax.install_neuronx_cc_hook()
    partition_name = (nc.partition_id_tensor.name
                      if nc.partition_id_tensor else None)
    in_names, out_names, out_avals, zero_outs = [], [], [], []
    for alloc in nc.m.functions[0].allocations:
        if not isinstance(alloc, mybir.MemoryLocationSet):
            continue
        name = alloc.memorylocations[0].name
        if alloc.kind == "ExternalInput":
            if name != partition_name:
                in_names.append(name)
        elif alloc.kind == "ExternalOutput":
            out_names.append(name)
            shape = tuple(alloc.tensor_shape)
            dtype = mybir.dt.np(alloc.dtype)
            out_avals.append(jax.core.ShapedArray(shape, dtype))
            zero_outs.append(_np.zeros(shape, dtype))
    n_params = len(in_names)
    all_names = in_names + out_names
    if partition_name is not None:
        all_names = all_names + [partition_name]

    def _body(*args):
        operands = list(args)
        if partition_name is not None:
            operands.append(bass2jax.partition_id_tensor())
        outs = bass2jax._bass_exec_p.bind(
            *operands,
            out_avals=tuple(out_avals),
            in_names=tuple(all_names),
            out_names=tuple(out_names),
            lowering_input_output_aliases=(),
            sim_require_finite=True,
            sim_require_nnan=True,
            nc=nc,
        )
        return tuple(outs)

    devices = jax.devices()[:NCORES]
    mesh = Mesh(_np.asarray(devices), ("core",))
    n_outs = len(out_names)
    sharded = jax.jit(
        shard_map(_body, mesh=mesh,
                  in_specs=(PartitionSpec("core"),) * (n_params + n_outs),
                  out_specs=(PartitionSpec("core"),) * n_outs,
                  check_rep=False),
        donate_argnums=tuple(range(n_params, n_params + n_outs)),
        keep_unused=True,
    )
    _CACHE["runner"] = (sharded, in_names, out_names, out_avals, zero_outs)
    return _CACHE["runner"]


def kernel(query, key, value, _run_kwargs=None):
    import numpy as _np
    nc = _get_nc()
    in_maps = make_in_maps(query, key, value)
    if _run_kwargs is not None:
        # profiling path for test.py
        from concourse.bass_utils import run_bass_kernel_spmd
        res = run_bass_kernel_spmd(nc, in_maps, list(range(NCORES)),
                                   **dict(_run_kwargs))
        _CACHE["last_result"] = res
        return assemble(res.results)
    sharded, in_names, out_names, out_avals, zero_outs = _get_runner(nc)
    concat_in = [
        _np.concatenate([m[name] for m in in_maps], axis=0)
        for name in in_names
    ]
    concat_zeros = [
        _np.zeros((NCORES * z.shape[0], *z.shape[1:]), z.dtype)
        for z in zero_outs
    ]
    out_arrs = sharded(*concat_in, *concat_zeros)
    results = [
        {name: _np.asarray(out_arrs[i]).reshape(NCORES, *out_avals[i].shape)[c]
         for i, name in enumerate(out_names)}
        for c in range(NCORES)
    ]
    return assemble(results)
